# revision 1
# baseline (speedup 1.0000x reference)
"""Causal self-attention (B=4, T=2048, C=1024, NH=16) on 8 TRN2 NeuronCores.

Sharding: core = 2*b + g  (b in 0..3 batches, g in 0..1 head-groups of 8 heads).
Each core computes the qkv projection for its 8 heads, causal flash attention,
and a partial output projection (rows g*512:(g+1)*512 of w_proj).  Host sums
the two partials per batch and adds b_proj.

v3 design (every matmul is K=128/M=128/N=512 so the PE HAM activity monitor
keeps the array at the warm 2.4 GHz clock):
  x^T     : x is DMA'd linearly and transposed on the PE in a prologue
            (DMA-transpose of bf16 measured ~40 GB/s — far too slow).
  qTp     : [2][m][128, T] bf16; rows hp*64..+64 hold head 2m+hp's q^T, the
            other 64 rows stay zero.  QK uses the shared kT[m] (both heads'
            dims) as stationary; the zero q rows null the other head.
  v_pad   : [128, kc, h, 128] bf16 = [64 v-dims | ones | 63 zeros]; PV output
            row 64 is the softmax denominator (ones-column trick).
  exp     : one ACT call per TWO key chunks ([128,1024] across 2 PSUM banks)
            to amortize the 352-cycle ACT startup; causal mask applied with
            one fused gpsimd affine_select per diagonal pair.
  softmax : reciprocal_approx_fast on [64,512] den tiles (rows 0/32), gpsimd
            partition broadcast, in-place bf16 multiply on unnormalized y^T.
  schedule: qkv(tb) / attention(qb) / proj(qb-1) interleaved; one shared
            [128,2,512] PSUM pool serves both the qkv pairs and the S pairs
            (4 banks) + 3 PV banks + 1 proj bank = 8.
"""

import numpy as np

import concourse.bass as bass
import concourse.mybir as mybir
import concourse.tile as tile
from concourse import bacc
from concourse.bass_utils import run_bass_kernel_spmd
from concourse.masks import make_identity

B, T, C = 4, 2048, 1024
NH, HD = 16, 64
G = 2              # head groups (cores per batch)
HPG = NH // G      # heads per group = 8
GD = HPG * HD      # dims per group = 512
N_CORES = B * G

FP32 = mybir.dt.float32
BF16 = mybir.dt.bfloat16

NCC = C // 128      # 8 contraction chunks for the qkv projection
NMB = GD // 128     # 4 blocks of 128 qkv-dims per section (head pairs)
NTB = T // 512      # 4 T-blocks of 512
NKC = T // 128      # 16 key chunks of 128


def build_nc():
    nc = bacc.Bacc()

    x = nc.declare_dram_parameter("x", [T, C], BF16, isOutput=False)
    wq = nc.declare_dram_parameter("wq", [C, GD], BF16, isOutput=False)
    wk = nc.declare_dram_parameter("wk", [C, GD], BF16, isOutput=False)
    wv = nc.declare_dram_parameter("wv", [C, GD], BF16, isOutput=False)
    bq = nc.declare_dram_parameter("bq", [GD], FP32, isOutput=False)
    bk = nc.declare_dram_parameter("bk", [GD], FP32, isOutput=False)
    bv = nc.declare_dram_parameter("bv", [GD], FP32, isOutput=False)
    wp = nc.declare_dram_parameter("wp", [GD, C], BF16, isOutput=False)
    ones = nc.declare_dram_parameter("ones", [128, HPG], BF16, isOutput=False)
    out = nc.declare_dram_parameter("out", [T, C], FP32, isOutput=True)

    from contextlib import ExitStack

    with tile.TileContext(nc) as tc, ExitStack() as stack:
        consts = stack.enter_context(tc.tile_pool(name="consts", bufs=1))
        persist = stack.enter_context(tc.tile_pool(name="persist", bufs=1))

        # ---- persistent activations ----
        qTp = [
            [persist.tile([128, T], BF16, tag=f"qTp{hp}{m}", name=f"qTp{hp}{m}")
             for m in range(NMB)]
            for hp in range(2)
        ]
        kT_t = [persist.tile([128, T], BF16, tag=f"kT{m}", name=f"kT{m}")
                for m in range(NMB)]
        yT_t = [persist.tile([128, T], BF16, tag=f"yT{m}", name=f"yT{m}")
                for m in range(NMB)]
        v_pad = persist.tile([128, NKC, HPG, 128], BF16, tag="v_pad", name="v_pad")
        xtc = [
            [persist.tile([128, 512], BF16, tag=f"xtc{tb}_{c}", name=f"xtc{tb}_{c}")
             for c in range(NCC)]
            for tb in range(NTB)
        ]
        wq_c = [persist.tile([128, GD], BF16, tag=f"wq{c}", name=f"wq{c}")
                for c in range(NCC)]
        wk_c = [persist.tile([128, GD], BF16, tag=f"wk{c}", name=f"wk{c}")
                for c in range(NCC)]
        wv_c = [persist.tile([128, GD], BF16, tag=f"wv{c}", name=f"wv{c}")
                for c in range(NCC)]
        wp_m = [persist.tile([128, C], BF16, tag=f"wp{m}", name=f"wp{m}")
                for m in range(NMB)]

        ident = consts.tile([128, 128], BF16, tag="ident")
        make_identity(nc, ident)

        # ---- zero-fill pad regions (overlaps the initial DMAs) ----
        for hp in range(2):
            zbase = (1 - hp) * 64
            for m in range(NMB):
                nc.gpsimd.memset(qTp[hp][m][zbase : zbase + 64, :], 0.0)
        nc.gpsimd.memset(v_pad[:, :, :, HD + 1 :], 0.0)

        # ---- tiny bias DMAs first (negligible bytes, needed early) ----
        bq_col = consts.tile([128, NMB], FP32, tag="bq_col")
        bk_col = consts.tile([128, NMB], FP32, tag="bk_col")
        for m in range(NMB):
            nc.sync.dma_start(out=bq_col[:, m : m + 1], in_=bq[bass.ts(m, 128)])
            nc.sync.dma_start(out=bk_col[:, m : m + 1], in_=bk[bass.ts(m, 128)])
        bv_bc = consts.tile([128, GD], FP32, tag="bv_bc")
        nc.sync.dma_start(out=bv_bc, in_=bv[None, :].partition_broadcast(128))

        # ---- prologue: linear-DMA x and transpose it on the PE; weight DMAs
        # interleave behind x's tb0 chunks so stage A can start at ~8us ----
        with (
            tc.tile_pool(name="xn", bufs=4) as xn_pool,
            tc.tile_pool(name="trps", bufs=4, space="PSUM") as trps_pool,
        ):
            for i in range(NKC):        # 128-row chunk of x
                tb, tsub = divmod(i, 4)
                xn = xn_pool.tile([128, C], BF16, tag="xn")
                nc.sync.dma_start(out=xn, in_=x[bass.ts(i, 128), :])
                for c in range(NCC):
                    trp = trps_pool.tile([128, 128], BF16, tag="trp")
                    nc.tensor.transpose(trp, xn[:, bass.ts(c, 128)], ident)
                    nc.vector.tensor_copy(xtc[tb][c][:, bass.ts(tsub, 128)], trp)
                if i == 3:      # x tb0 issued; now queue the weights
                    for c in range(NCC):
                        nc.sync.dma_start(out=wq_c[c], in_=wq[bass.ts(c, 128), :])
                    for c in range(NCC):
                        nc.sync.dma_start(out=wk_c[c], in_=wk[bass.ts(c, 128), :])
                    for c in range(NCC):
                        nc.sync.dma_start(out=wv_c[c], in_=wv[bass.ts(c, 128), :])
        for m in range(NMB):
            nc.sync.dma_start(out=wp_m[m], in_=wp[bass.ts(m, 128), :])
        # ones column of v_pad (after the memset in program order)
        for kc in range(NKC):
            nc.sync.dma_start(out=v_pad[:, kc, :, HD : HD + 1], in_=ones[:, :, None])

        # ---- main pools: shared 2-bank pair pool + PV + proj ----
        with (
            tc.tile_pool(name="pT", bufs=6) as pT_pool,
            tc.tile_pool(name="rec", bufs=2) as rec_pool,
            tc.tile_pool(name="rbc", bufs=2) as rbc_pool,
            tc.tile_pool(name="osb", bufs=2) as osb_pool,
            tc.tile_pool(name="big", bufs=2, space="PSUM") as big_pool,
            tc.tile_pool(name="pvps", bufs=4, space="PSUM") as pvps_pool,
        ):
            def emit_qkv(tb):
                tcols = bass.ts(tb, 512)
                for m in range(NMB):
                    ps = big_pool.tile([128, 2, 512], FP32, tag="big")
                    for c in range(NCC):
                        nc.tensor.matmul(
                            ps[:, 0, :], wq_c[c][:, bass.ts(m, 128)], xtc[tb][c],
                            start=(c == 0), stop=(c == NCC - 1),
                        )
                    for c in range(NCC):
                        nc.tensor.matmul(
                            ps[:, 1, :], wk_c[c][:, bass.ts(m, 128)], xtc[tb][c],
                            start=(c == 0), stop=(c == NCC - 1),
                        )
                    nc.vector.tensor_scalar_add(
                        qTp[0][m][0:64, tcols], ps[0:64, 0, :], bq_col[0:64, m : m + 1]
                    )
                    nc.vector.tensor_scalar_add(
                        qTp[1][m][64:128, tcols], ps[64:128, 0, :],
                        bq_col[64:128, m : m + 1],
                    )
                    nc.vector.tensor_scalar_add(
                        kT_t[m][:, tcols], ps[:, 1, :], bk_col[:, m : m + 1]
                    )
                for tp in range(2):     # tsub pairs
                    ps = big_pool.tile([128, 2, 512], FP32, tag="big")
                    for j in range(2):
                        tsub = 2 * tp + j
                        for c in range(NCC):
                            nc.tensor.matmul(
                                ps[:, j, :],
                                xtc[tb][c][:, bass.ts(tsub, 128)], wv_c[c],
                                start=(c == 0), stop=(c == NCC - 1),
                            )
                    for j in range(2):
                        kc = tb * 4 + 2 * tp + j
                        vt = v_pad[:, kc, :, :]
                        nc.vector.tensor_add(
                            vt[:, :, 0:HD],
                            ps[:, j, :].rearrange("p (h d) -> p h d", h=HPG),
                            bv_bc.rearrange("p (h d) -> p h d", h=HPG),
                        )

            dens = {}

            def emit_attention(qb):
                kcmax = (qb + 1) * 4
                for m in range(NMB):
                    for hp in range(2):
                        pv = pvps_pool.tile([128, 512], FP32, tag="pvps", name="pvps")
                        for p in range(kcmax // 2):
                            r0 = 2 * p - qb * 4
                            # queries below the diagonal band are fully masked:
                            # only the last w of the 512 queries participate
                            w = 512 - 128 * r0 if r0 > 0 else 512
                            q0 = qb * 512 + (512 - w)
                            sp = big_pool.tile([128, 2, 512], FP32, tag="big")
                            for j in range(2):
                                kc = 2 * p + j
                                nc.tensor.matmul(
                                    sp[:, j, 0:w],
                                    kT_t[m][:, bass.ts(kc, 128)],
                                    qTp[hp][m][:, q0 : q0 + w],
                                    start=True, stop=True,
                                )
                            pT2 = pT_pool.tile([128, 2, 512], BF16, tag="pT2")
                            nc.scalar.activation(
                                out=pT2[:, :, 512 - w :],
                                in_=sp[:, :, 0:w],
                                func=mybir.ActivationFunctionType.Exp,
                                scale=1.0 / float(np.sqrt(HD)),
                            )
                            if r0 >= 0:
                                # keep key<=query on both halves at once:
                                # iota = f - il - 128*j   (f = query - 128*r0)
                                nc.gpsimd.affine_select(
                                    out=pT2[:, :, 512 - w :],
                                    in_=pT2[:, :, 512 - w :],
                                    compare_op=mybir.AluOpType.is_ge,
                                    fill=0.0,
                                    base=0,
                                    channel_multiplier=-1,
                                    pattern=[[-128, 2], [1, w]],
                                )
                            for j in range(2):
                                kc = 2 * p + j
                                nc.tensor.matmul(
                                    pv[:, 512 - w :],
                                    v_pad[:, kc, 2 * m + hp, :],
                                    pT2[:, j, 512 - w :],
                                    start=(kc == 0), stop=(kc == kcmax - 1),
                                )
                        if qb == 0 and hp == 0:
                            dens[m] = rec_pool.tile(
                                [64, 512], FP32, tag=f"den{m}", name=f"den{m}", bufs=1
                            )
                            nc.vector.memset(dens[m], 1.0)
                        # unnormalized y^T (bf16) + denominator row (at 0 / 32)
                        nc.vector.tensor_copy(
                            yT_t[m][bass.ts(hp, 64), bass.ts(qb, 512)], pv[0:HD, :]
                        )
                        nc.vector.tensor_copy(
                            dens[m][32 * hp : 32 * hp + 1, :], pv[HD : HD + 1, :]
                        )
                # normalize: cheap approx reciprocal, broadcast, in-place multiply
                for m in range(NMB):
                    denr = rec_pool.tile([64, 512], FP32, tag="denr", name="denr")
                    nc.vector.reciprocal_approx_fast(denr, dens[m])
                    for hp in range(2):
                        if hp == 0:
                            src_row = denr[0:1, :]
                        else:
                            dtmp = rec_pool.tile([1, 512], FP32, tag="dtmp", name="dtmp")
                            nc.vector.tensor_copy(dtmp, denr[32:33, :])
                            src_row = dtmp
                        rbc = rbc_pool.tile([128, 512], FP32, tag="rbc", name="rbc")
                        nc.gpsimd.partition_broadcast(rbc, src_row)
                        yt = yT_t[m][bass.ts(hp, 64), bass.ts(qb, 512)]
                        nc.vector.tensor_mul(yt, yt, rbc[bass.ts(hp, 64), :])

            def emit_proj(qb):
                for tsub in range(4):
                    tb16 = qb * 4 + tsub
                    ps = big_pool.tile([128, 2, 512], FP32, tag="big")
                    for nb in range(C // 512):
                        for c in range(NMB):
                            nc.tensor.matmul(
                                ps[:, nb, :],
                                yT_t[c][:, bass.ts(tb16, 128)],
                                wp_m[c][:, bass.ts(nb, 512)],
                                start=(c == 0), stop=(c == NMB - 1),
                            )
                    for nb in range(C // 512):
                        osb = osb_pool.tile([128, 512], FP32, tag="osb")
                        nc.vector.tensor_copy(osb, ps[:, nb, :])
                        nc.sync.dma_start(
                            out=out[bass.ts(tb16, 128), bass.ts(nb, 512)], in_=osb
                        )

            # interleave: qkv(tb) feeds attention(qb=tb); proj lags one block
            emit_qkv(0)
            emit_attention(0)
            emit_qkv(1)
            emit_attention(1)
            emit_proj(0)
            emit_qkv(2)
            emit_attention(2)
            emit_proj(1)
            emit_qkv(3)
            emit_attention(3)
            emit_proj(2)
            emit_proj(3)

    nc.compile()
    return nc


_CACHE = {}


def _get_nc():
    if "nc" not in _CACHE:
        _CACHE["nc"] = build_nc()
    return _CACHE["nc"]


def _to_bf16(a):
    import ml_dtypes

    a = np.asarray(a, dtype=np.float32)
    return np.ascontiguousarray(a.astype(ml_dtypes.bfloat16))


def make_in_maps(x, w_qkv, b_qkv, w_proj):
    x = np.asarray(x, dtype=np.float32)
    w_qkv = np.asarray(w_qkv, dtype=np.float32)
    b_qkv = np.asarray(b_qkv, dtype=np.float32)
    in_maps = []
    for core in range(N_CORES):
        b, g = divmod(core, G)
        in_maps.append(
            {
                "x": _to_bf16(x[b]),
                "wq": _to_bf16(w_qkv[:, GD * g : GD * g + GD]),
                "wk": _to_bf16(w_qkv[:, C + GD * g : C + GD * g + GD]),
                "wv": _to_bf16(w_qkv[:, 2 * C + GD * g : 2 * C + GD * g + GD]),
                "bq": np.ascontiguousarray(b_qkv[GD * g : GD * g + GD]),
                "bk": np.ascontiguousarray(b_qkv[C + GD * g : C + GD * g + GD]),
                "bv": np.ascontiguousarray(b_qkv[2 * C + GD * g : 2 * C + GD * g + GD]),
                "wp": _to_bf16(np.asarray(w_proj, dtype=np.float32)[GD * g : GD * g + GD, :]),
                "ones": _to_bf16(np.ones((128, HPG), dtype=np.float32)),
            }
        )
    return in_maps


def _assemble(results, b_proj):
    y = np.empty((B, T, C), dtype=np.float32)
    for b in range(B):
        y[b] = results[G * b]["out"] + results[G * b + 1]["out"]
    y += np.asarray(b_proj, dtype=np.float32)[None, None, :]
    return y


def kernel(x, w_qkv, b_qkv, w_proj, b_proj):
    nc = _get_nc()
    in_maps = make_in_maps(x, w_qkv, b_qkv, w_proj)
    res = run_bass_kernel_spmd(nc, in_maps, list(range(N_CORES)))
    return _assemble(res.results, b_proj)



# revision 4
# speedup vs baseline: 1.4348x; 1.4348x over previous
"""Causal self-attention (B=4, T=2048, C=1024, NH=16) on 8 TRN2 NeuronCores.

Sharding: core = 2*b + g  (b in 0..3 batches, g in 0..1 head-groups of 8 heads).
Each core computes the qkv projection for its 8 heads, causal flash attention,
and a partial output projection (rows g*512:(g+1)*512 of w_proj).  Host sums
the two partials per batch and adds b_proj.

v4 design (PE-roofline focused; ~240us of bf16 matmul work per core):
  xT      : host passes x^T (bf16) so no on-device transposes; one big tile
            xt_all[128, 8, 2048] is DMA'd directly (tb0 columns first).
  DMA     : weights go on the Activation HWDGE queue, x/out on the SP queue
            (two parallel hardware queues; consolidated rearranged DMAs).
  qTp     : [2][m][128, T] bf16; rows hp*64..+64 hold head 2m+hp's q^T, the
            other 64 rows zero.  QK uses the shared kT[m] (both heads' dims)
            as stationary; the zero q rows null the other head.
  mask    : causal mask applied PRE-exp by a tiny extra matmul per diagonal
            chunk: stationary=identity, moving=tri_neg ([128,128] const with
            -240 strictly below the diagonal) accumulated into the S psum.
            exp((-240)/8) ~ 1e-13 -> harmless.  No gpsimd in the chain.
  exp     : one ACT call per key-chunk PAIR ([128,2,512] across 2 PSUM banks)
            to amortize the ~352-cycle ACT startup.
  pipeline: per (qb, m) the two head chains hp0/hp1 alternate
            QK(A,p) exp(A) PV(B,p-1) QK(B,p) exp(B) PV(A,p-1), with qkv/proj
            "filler units" pumped between steps, so the PE never waits on the
            scalar engine and stays at the warm 2.4 GHz clock.
  v_pad   : [128, kc, h, 65] bf16 = [64 v-dims | ones]; PV output row 64 is
            the softmax denominator (ones-column trick).
  norm    : unnormalized y^T (bf16) scaled at the end of each (qb, m) by
            reciprocal_approx_fast + gpsimd partition broadcast.
  schedule: qkv(0); attn(0)+qkv(1); attn(1)+qkv(2); attn(2)+qkv(3);
            attn(3)+proj(0..2); proj(3).
"""

import numpy as np

import concourse.bass as bass
import concourse.mybir as mybir
import concourse.tile as tile
from concourse import bacc
from concourse.bass_utils import run_bass_kernel_spmd
from concourse.masks import make_identity

B, T, C = 4, 2048, 1024
NH, HD = 16, 64
G = 2              # head groups (cores per batch)
HPG = NH // G      # heads per group = 8
GD = HPG * HD      # dims per group = 512
N_CORES = B * G

FP32 = mybir.dt.float32
BF16 = mybir.dt.bfloat16

NCC = C // 128      # 8 contraction chunks for the qkv projection
NMB = GD // 128     # 4 blocks of 128 qkv-dims per section (head pairs)
NTB = T // 512      # 4 T-blocks of 512
NKC = T // 128      # 16 key chunks of 128
VP = HD + 1         # v_pad columns: 64 v-dims + ones column


def build_nc():
    nc = bacc.Bacc()

    xT = nc.declare_dram_parameter("xT", [C, T], BF16, isOutput=False)
    wq = nc.declare_dram_parameter("wq", [C, GD], BF16, isOutput=False)
    wk = nc.declare_dram_parameter("wk", [C, GD], BF16, isOutput=False)
    wv = nc.declare_dram_parameter("wv", [C, GD], BF16, isOutput=False)
    bq = nc.declare_dram_parameter("bq", [GD], FP32, isOutput=False)
    bk = nc.declare_dram_parameter("bk", [GD], FP32, isOutput=False)
    bv = nc.declare_dram_parameter("bv", [GD], FP32, isOutput=False)
    wp = nc.declare_dram_parameter("wp", [GD, C], BF16, isOutput=False)
    out = nc.declare_dram_parameter("out", [T, C], FP32, isOutput=True)

    from contextlib import ExitStack

    with tile.TileContext(nc) as tc, ExitStack() as stack:
        consts = stack.enter_context(tc.tile_pool(name="consts", bufs=1))
        persist = stack.enter_context(tc.tile_pool(name="persist", bufs=1))

        # ---- persistent activations / weights ----
        qTp = [
            [persist.tile([128, T], BF16, tag=f"qTp{hp}{m}", name=f"qTp{hp}{m}")
             for m in range(NMB)]
            for hp in range(2)
        ]
        kT_t = [persist.tile([128, T], BF16, tag=f"kT{m}", name=f"kT{m}")
                for m in range(NMB)]
        yT_t = [persist.tile([128, T], BF16, tag=f"yT{m}", name=f"yT{m}")
                for m in range(NMB)]
        v_pad = persist.tile([128, NKC, HPG, VP], BF16, tag="v_pad", name="v_pad")
        xt_all = persist.tile([128, NCC, T], BF16, tag="xt_all", name="xt_all")
        wq_all = persist.tile([128, NCC, GD], BF16, tag="wq_all", name="wq_all")
        wk_all = persist.tile([128, NCC, GD], BF16, tag="wk_all", name="wk_all")
        wv_all = persist.tile([128, NCC, GD], BF16, tag="wv_all", name="wv_all")
        wp_all = persist.tile([128, NMB, C], BF16, tag="wp_all", name="wp_all")

        ident = consts.tile([128, 128], BF16, tag="ident")
        make_identity(nc, ident)
        # tri_neg[k, i] = -240 where i < k (strictly below diagonal), else 0
        tri_neg = consts.tile([128, 128], BF16, tag="tri_neg")
        nc.gpsimd.memset(tri_neg, -240.0)
        nc.gpsimd.affine_select(
            out=tri_neg, in_=tri_neg,
            pattern=[[-1, 128]],
            compare_op=mybir.AluOpType.is_ge,
            fill=0.0, base=-1, channel_multiplier=1,
        )

        # ---- zero-fill the dead half of each qTp; ones column of v_pad ----
        for hp in range(2):
            zbase = (1 - hp) * 64
            for m in range(NMB):
                nc.gpsimd.memset(qTp[hp][m][zbase: zbase + 64, :], 0.0)
        nc.gpsimd.memset(v_pad[:, :, :, HD: HD + 1], 1.0)

        # ---- input DMAs: weights/bias on Act queue, x on SP queue ----
        bq_col = consts.tile([128, NMB], FP32, tag="bq_col")
        bk_col = consts.tile([128, NMB], FP32, tag="bk_col")
        bv_bc = consts.tile([128, GD], FP32, tag="bv_bc")
        nc.scalar.dma_start(out=bq_col, in_=bq.rearrange("(m p) -> p m", p=128))
        nc.scalar.dma_start(out=bk_col, in_=bk.rearrange("(m p) -> p m", p=128))
        nc.scalar.dma_start(out=bv_bc, in_=bv[None, :].partition_broadcast(128))
        nc.scalar.dma_start(out=wq_all, in_=wq.rearrange("(c d) n -> d c n", d=128))
        nc.scalar.dma_start(out=wk_all, in_=wk.rearrange("(c d) n -> d c n", d=128))
        nc.scalar.dma_start(out=wv_all, in_=wv.rearrange("(c d) n -> d c n", d=128))
        nc.scalar.dma_start(out=wp_all, in_=wp.rearrange("(c d) n -> d c n", d=128))
        xt_src = xT.rearrange("(c d) t -> d c t", d=128)
        nc.sync.dma_start(out=xt_all[:, :, 0:512], in_=xt_src[:, :, 0:512])
        nc.sync.dma_start(out=xt_all[:, :, 512:T], in_=xt_src[:, :, 512:T])

        with (
            tc.tile_pool(name="pt", bufs=4) as ptpool,
            tc.tile_pool(name="dn", bufs=2) as dnpool,
            tc.tile_pool(name="rbc", bufs=2) as rbcpool,
            tc.tile_pool(name="osb", bufs=2) as osbpool,
            tc.tile_pool(name="sp", bufs=2, space="PSUM") as spool,
            tc.tile_pool(name="pv", bufs=2, space="PSUM") as pvpool,
            tc.tile_pool(name="fps", bufs=2, space="PSUM") as fpool,
        ):
            # ---------------- filler units (qkv projection / out proj) -----
            def q_unit(tb, m):
                def emit():
                    ps = fpool.tile([128, 512], FP32, tag="fps", name="fps")
                    for c in range(NCC):
                        nc.tensor.matmul(
                            ps, wq_all[:, c, bass.ts(m, 128)],
                            xt_all[:, c, bass.ts(tb, 512)],
                            start=(c == 0), stop=(c == NCC - 1),
                        )
                    tcols = bass.ts(tb, 512)
                    nc.vector.tensor_scalar_add(
                        qTp[0][m][0:64, tcols], ps[0:64, :], bq_col[0:64, m: m + 1]
                    )
                    nc.vector.tensor_scalar_add(
                        qTp[1][m][64:128, tcols], ps[64:128, :],
                        bq_col[64:128, m: m + 1],
                    )
                return emit

            def k_unit(tb, m):
                def emit():
                    ps = fpool.tile([128, 512], FP32, tag="fps", name="fps")
                    for c in range(NCC):
                        nc.tensor.matmul(
                            ps, wk_all[:, c, bass.ts(m, 128)],
                            xt_all[:, c, bass.ts(tb, 512)],
                            start=(c == 0), stop=(c == NCC - 1),
                        )
                    nc.vector.tensor_scalar_add(
                        kT_t[m][:, bass.ts(tb, 512)], ps, bk_col[:, m: m + 1]
                    )
                return emit

            def v_unit(tb, tsub):
                def emit():
                    kc = tb * 4 + tsub
                    ps = fpool.tile([128, 512], FP32, tag="fps", name="fps")
                    for c in range(NCC):
                        nc.tensor.matmul(
                            ps, xt_all[:, c, bass.ts(kc, 128)], wv_all[:, c, :],
                            start=(c == 0), stop=(c == NCC - 1),
                        )
                    nc.vector.tensor_add(
                        v_pad[:, kc, :, 0:HD],
                        ps.rearrange("p (h d) -> p h d", h=HPG),
                        bv_bc.rearrange("p (h d) -> p h d", h=HPG),
                    )
                return emit

            proj_osb = {}

            def proj_unit(qb, tsub, nb):
                def emit():
                    tb16 = qb * 4 + tsub
                    ps = fpool.tile([128, 512], FP32, tag="fps", name="fps")
                    for c in range(NMB):
                        nc.tensor.matmul(
                            ps, yT_t[c][:, bass.ts(tb16, 128)],
                            wp_all[:, c, bass.ts(nb, 512)],
                            start=(c == 0), stop=(c == NMB - 1),
                        )
                    if nb == 0:
                        proj_osb[tb16] = osbpool.tile([128, C], FP32, tag="osb", name="osb")
                    osb = proj_osb[tb16]
                    nc.vector.tensor_copy(osb[:, bass.ts(nb, 512)], ps)
                    if nb == 1:
                        nc.sync.dma_start(
                            out=out[bass.ts(tb16, 128), :], in_=osb
                        )
                return emit

            def qkv_units(tb):
                us = []
                for m in range(NMB):
                    us.append(q_unit(tb, m))
                for m in range(NMB):
                    us.append(k_unit(tb, m))
                for tsub in range(4):
                    us.append(v_unit(tb, tsub))
                return us

            def proj_units(qb):
                return [proj_unit(qb, tsub, nb)
                        for tsub in range(4) for nb in range(2)]

            # ---------------- attention window -----------------------------
            def chunk_geom(qb, kc):
                c_off = kc - 4 * qb
                w = 512 if c_off < 0 else 512 - 128 * c_off
                return c_off, 512 - w          # (diag offset, first live col)

            def attn_window(qb, units):
                P = 2 * (qb + 1)
                kcmax = 4 * (qb + 1)
                n_hs = NMB * P * 2             # halfsteps in this window
                state = {"u": 0, "hs": 0}

                def pump():
                    state["hs"] += 1
                    while (state["u"] < len(units)
                           and state["u"] * n_hs < state["hs"] * len(units)):
                        units[state["u"]]()
                        state["u"] += 1

                for m in range(NMB):
                    pv = [pvpool.tile([128, 512], FP32, tag="pv", name=f"pv{_hp}")
                          for _hp in range(2)]

                    def qk_exp(hp, p):
                        S = spool.tile([128, 2, 512], FP32, tag="sp", name="S")
                        pcol0 = 512
                        for j in range(2):
                            kc = 2 * p + j
                            c_off, col0 = chunk_geom(qb, kc)
                            pcol0 = min(pcol0, col0)
                            diag = c_off >= 0
                            nc.tensor.matmul(
                                S[:, j, col0:512],
                                kT_t[m][:, bass.ts(kc, 128)],
                                qTp[hp][m][:, qb * 512 + col0: qb * 512 + 512],
                                start=True, stop=not diag,
                            )
                            if diag:
                                nc.tensor.matmul(
                                    S[:, j, col0: col0 + 128],
                                    ident, tri_neg,
                                    start=False, stop=True,
                                )
                        pt = ptpool.tile([128, 2, 512], BF16, tag="pt", name="pt")
                        nc.scalar.activation(
                            out=pt[:, :, pcol0:512],
                            in_=S[:, :, pcol0:512],
                            func=mybir.ActivationFunctionType.Exp,
                            scale=1.0 / float(np.sqrt(HD)),
                        )
                        return pt

                    def emit_pv(hp, p, pt):
                        for j in range(2):
                            kc = 2 * p + j
                            _, col0 = chunk_geom(qb, kc)
                            nc.tensor.matmul(
                                pv[hp][0:VP, col0:512],
                                v_pad[:, kc, 2 * m + hp, :],
                                pt[:, j, col0:512],
                                start=(kc == 0), stop=(kc == kcmax - 1),
                            )

                    prev = [None, None]
                    for p in range(P):
                        pt_a = qk_exp(0, p)
                        if p > 0:
                            emit_pv(1, p - 1, prev[1])
                        pump()
                        pt_b = qk_exp(1, p)
                        if p > 0:
                            emit_pv(0, p - 1, prev[0])
                        pump()
                        prev = [pt_a, pt_b]
                    emit_pv(0, P - 1, prev[0])
                    emit_pv(1, P - 1, prev[1])

                    # drain + normalize this m-block
                    for hp in range(2):
                        nc.vector.tensor_copy(
                            yT_t[m][bass.ts(hp, 64), bass.ts(qb, 512)],
                            pv[hp][0:HD, :],
                        )
                        den = dnpool.tile([1, 512], FP32, tag=f"den{hp}", name=f"den{hp}")
                        nc.vector.tensor_copy(den, pv[hp][HD: HD + 1, :])
                        denr = dnpool.tile([1, 512], FP32, tag=f"denr{hp}", name=f"denr{hp}")
                        nc.vector.reciprocal_approx_fast(denr, den)
                        rbc = rbcpool.tile([128, 512], FP32, tag="rbc", name="rbc")
                        nc.gpsimd.partition_broadcast(rbc, denr)
                        yt = yT_t[m][bass.ts(hp, 64), bass.ts(qb, 512)]
                        nc.vector.tensor_mul(yt, yt, rbc[bass.ts(hp, 64), :])

                # flush any filler not yet emitted
                while state["u"] < len(units):
                    units[state["u"]]()
                    state["u"] += 1

            # ---------------- schedule -------------------------------------
            for u in qkv_units(0):
                u()
            attn_window(0, qkv_units(1))
            attn_window(1, qkv_units(2))
            attn_window(2, qkv_units(3))
            attn_window(3, proj_units(0) + proj_units(1) + proj_units(2))
            for u in proj_units(3):
                u()

    nc.compile()
    return nc


_CACHE = {}


def _get_nc():
    if "nc" not in _CACHE:
        _CACHE["nc"] = build_nc()
    return _CACHE["nc"]


def _to_bf16(a):
    import ml_dtypes

    a = np.asarray(a, dtype=np.float32)
    return np.ascontiguousarray(a.astype(ml_dtypes.bfloat16))


def make_in_maps(x, w_qkv, b_qkv, w_proj):
    x = np.asarray(x, dtype=np.float32)
    w_qkv = np.asarray(w_qkv, dtype=np.float32)
    b_qkv = np.asarray(b_qkv, dtype=np.float32)
    w_proj = np.asarray(w_proj, dtype=np.float32)
    xTs = [_to_bf16(x[b].T) for b in range(B)]
    in_maps = []
    for core in range(N_CORES):
        b, g = divmod(core, G)
        gs = slice(GD * g, GD * g + GD)
        in_maps.append(
            {
                "xT": xTs[b],
                "wq": _to_bf16(w_qkv[:, gs]),
                "wk": _to_bf16(w_qkv[:, C + GD * g: C + GD * g + GD]),
                "wv": _to_bf16(w_qkv[:, 2 * C + GD * g: 2 * C + GD * g + GD]),
                "bq": np.ascontiguousarray(b_qkv[gs]),
                "bk": np.ascontiguousarray(b_qkv[C + GD * g: C + GD * g + GD]),
                "bv": np.ascontiguousarray(b_qkv[2 * C + GD * g: 2 * C + GD * g + GD]),
                "wp": _to_bf16(w_proj[gs, :]),
            }
        )
    return in_maps


def _assemble(results, b_proj):
    y = np.empty((B, T, C), dtype=np.float32)
    for b in range(B):
        y[b] = results[G * b]["out"] + results[G * b + 1]["out"]
    y += np.asarray(b_proj, dtype=np.float32)[None, None, :]
    return y


def kernel(x, w_qkv, b_qkv, w_proj, b_proj):
    nc = _get_nc()
    in_maps = make_in_maps(x, w_qkv, b_qkv, w_proj)
    res = run_bass_kernel_spmd(nc, in_maps, list(range(N_CORES)))
    return _assemble(res.results, b_proj)


# revision 5
# speedup vs baseline: 1.4775x; 1.0297x over previous
"""Causal self-attention (B=4, T=2048, C=1024, NH=16) on 8 TRN2 NeuronCores.

Sharding: core = 2*b + g  (b in 0..3 batches, g in 0..1 head-groups of 8 heads).
Each core computes the qkv projection for its 8 heads, causal flash attention,
and a partial output projection (rows g*512:(g+1)*512 of w_proj).  Host sums
the two partials per batch and adds b_proj.

v5 design (PE-roofline focused; ~240us of bf16 matmul work per core):
  layout  : host pre-rearranges every input into its SBUF tile layout
            (x^T tiled [128, tb, c, 512], weights [128, c, n]) so each DMA is
            one instruction with 4KB contiguous packets at full HBM rate.
            Weights ride the Activation HWDGE queue, x/out the SP queue.
  qTp     : [2][m][128, T] bf16; rows hp*64..+64 hold head 2m+hp's q^T, the
            other 64 rows zero.  QK uses the shared kT[m] (both heads' dims)
            as stationary; the zero q rows null the other head.
  mask    : causal mask applied PRE-exp by a tiny extra matmul per diagonal
            chunk: stationary=identity, moving=tri_neg ([128,128] const with
            -240 strictly below the diagonal) accumulated into the S psum.
            exp(-240/8) ~ 1e-13 -> harmless.  No gpsimd in the softmax chain.
  exp     : one ACT call per key-chunk PAIR ([128,2,512] across 2 PSUM banks)
            to amortize the ~352-cycle ACT startup.
  pipeline: per (qb, m) the two head chains hp0/hp1 alternate
            QK(A,p) exp(A) PV(B,p-1) QK(B,p) exp(B) PV(A,p-1), with qkv/proj
            "filler units" pumped between steps, so the PE never waits on the
            scalar engine and stays at the warm 2.4 GHz clock.
  v_pad   : [128, kc, h, 65] bf16 = [64 v-dims | ones]; PV output row 64 is
            the softmax denominator (ones-column trick).
  norm    : unnormalized y^T (bf16) scaled at the end of each (qb, m) by
            reciprocal_approx_fast + gpsimd partition broadcast.
  schedule: qkv(0); attn(0)+qkv(1); attn(1)+qkv(2); then windows 2 and 3 are
            MERGED - m-groups of qb=2 and qb=3 interleave (qb=3 only needs
            its own q projection early, which is force-flushed) so the
            scalar-engine exp load stays ~80% instead of peaking at ~98%;
            filler = qkv(3)+proj(0..2); tail = proj(3).
"""

import numpy as np

import concourse.bass as bass
import concourse.mybir as mybir
import concourse.tile as tile
from concourse import bacc
from concourse.bass_utils import run_bass_kernel_spmd
from concourse.masks import make_identity

B, T, C = 4, 2048, 1024
NH, HD = 16, 64
G = 2              # head groups (cores per batch)
HPG = NH // G      # heads per group = 8
GD = HPG * HD      # dims per group = 512
N_CORES = B * G

FP32 = mybir.dt.float32
BF16 = mybir.dt.bfloat16

NCC = C // 128      # 8 contraction chunks for the qkv projection
NMB = GD // 128     # 4 blocks of 128 qkv-dims per section (head pairs)
NTB = T // 512      # 4 T-blocks of 512
NKC = T // 128      # 16 key chunks of 128
VP = HD + 1         # v_pad columns: 64 v-dims + ones column


def build_nc():
    nc = bacc.Bacc()

    xt4 = nc.declare_dram_parameter("xt4", [128, NTB, NCC, 512], BF16, isOutput=False)
    wq = nc.declare_dram_parameter("wq", [128, NCC, GD], BF16, isOutput=False)
    wk = nc.declare_dram_parameter("wk", [128, NCC, GD], BF16, isOutput=False)
    wv = nc.declare_dram_parameter("wv", [128, NCC, GD], BF16, isOutput=False)
    bqc = nc.declare_dram_parameter("bqc", [128, NMB], FP32, isOutput=False)
    bkc = nc.declare_dram_parameter("bkc", [128, NMB], FP32, isOutput=False)
    bvb = nc.declare_dram_parameter("bvb", [128, GD], FP32, isOutput=False)
    wp = nc.declare_dram_parameter("wp", [128, NMB, C], BF16, isOutput=False)
    out = nc.declare_dram_parameter("out", [T, C], FP32, isOutput=True)

    from contextlib import ExitStack

    with tile.TileContext(nc) as tc, ExitStack() as stack:
        consts = stack.enter_context(tc.tile_pool(name="consts", bufs=1))
        persist = stack.enter_context(tc.tile_pool(name="persist", bufs=1))

        # ---- persistent activations / weights ----
        qTp = [
            [persist.tile([128, T], BF16, tag=f"qTp{hp}{m}", name=f"qTp{hp}{m}")
             for m in range(NMB)]
            for hp in range(2)
        ]
        kT_t = [persist.tile([128, T], BF16, tag=f"kT{m}", name=f"kT{m}")
                for m in range(NMB)]
        yT_t = [persist.tile([128, T], BF16, tag=f"yT{m}", name=f"yT{m}")
                for m in range(NMB)]
        v_pad = persist.tile([128, NKC, HPG, VP], BF16, tag="v_pad", name="v_pad")
        xt_all = persist.tile([128, NTB, NCC, 512], BF16, tag="xt_all", name="xt_all")
        wq_all = persist.tile([128, NCC, GD], BF16, tag="wq_all", name="wq_all")
        wk_all = persist.tile([128, NCC, GD], BF16, tag="wk_all", name="wk_all")
        wv_all = persist.tile([128, NCC, GD], BF16, tag="wv_all", name="wv_all")
        wp_all = persist.tile([128, NMB, C], BF16, tag="wp_all", name="wp_all")

        ident = consts.tile([128, 128], BF16, tag="ident")
        make_identity(nc, ident)
        # tri_neg[k, i] = -240 where i < k (strictly below diagonal), else 0
        tri_neg = consts.tile([128, 128], BF16, tag="tri_neg")
        nc.gpsimd.memset(tri_neg, -240.0)
        nc.gpsimd.affine_select(
            out=tri_neg, in_=tri_neg,
            pattern=[[-1, 128]],
            compare_op=mybir.AluOpType.is_ge,
            fill=0.0, base=-1, channel_multiplier=1,
        )

        # ---- zero-fill the dead half of each qTp; ones column of v_pad ----
        for hp in range(2):
            zbase = (1 - hp) * 64
            for m in range(NMB):
                nc.gpsimd.memset(qTp[hp][m][zbase: zbase + 64, :], 0.0)
        nc.gpsimd.memset(v_pad[:, :, :, HD: HD + 1], 1.0)

        # ---- input DMAs: weights/bias on Act queue, x on SP queue ----
        bq_col = consts.tile([128, NMB], FP32, tag="bq_col")
        bk_col = consts.tile([128, NMB], FP32, tag="bk_col")
        bv_bc = consts.tile([128, GD], FP32, tag="bv_bc")
        nc.scalar.dma_start(out=wq_all, in_=wq[:, :, :])
        nc.scalar.dma_start(out=bq_col, in_=bqc[:, :])
        nc.scalar.dma_start(out=bk_col, in_=bkc[:, :])
        nc.scalar.dma_start(out=bv_bc, in_=bvb[:, :])
        nc.scalar.dma_start(out=wk_all, in_=wk[:, :, :])
        nc.scalar.dma_start(out=wv_all, in_=wv[:, :, :])
        nc.scalar.dma_start(out=wp_all, in_=wp[:, :, :])
        for tb in range(NTB):
            nc.sync.dma_start(out=xt_all[:, tb], in_=xt4[:, tb])

        with (
            tc.tile_pool(name="pt", bufs=4) as ptpool,
            tc.tile_pool(name="dn", bufs=2) as dnpool,
            tc.tile_pool(name="rbc", bufs=2) as rbcpool,
            tc.tile_pool(name="osb", bufs=2) as osbpool,
            tc.tile_pool(name="sp", bufs=2, space="PSUM") as spool,
            tc.tile_pool(name="pv", bufs=2, space="PSUM") as pvpool,
            tc.tile_pool(name="fps", bufs=2, space="PSUM") as fpool,
        ):
            # ---------------- filler units (qkv projection / out proj) -----
            def q_unit(tb, m):
                def emit():
                    ps = fpool.tile([128, 512], FP32, tag="fps", name="fps")
                    for c in range(NCC):
                        nc.tensor.matmul(
                            ps, wq_all[:, c, bass.ts(m, 128)],
                            xt_all[:, tb, c, :],
                            start=(c == 0), stop=(c == NCC - 1),
                        )
                    tcols = bass.ts(tb, 512)
                    nc.vector.tensor_scalar_add(
                        qTp[0][m][0:64, tcols], ps[0:64, :], bq_col[0:64, m: m + 1]
                    )
                    nc.vector.tensor_scalar_add(
                        qTp[1][m][64:128, tcols], ps[64:128, :],
                        bq_col[64:128, m: m + 1],
                    )
                return emit

            def k_unit(tb, m):
                def emit():
                    ps = fpool.tile([128, 512], FP32, tag="fps", name="fps")
                    for c in range(NCC):
                        nc.tensor.matmul(
                            ps, wk_all[:, c, bass.ts(m, 128)],
                            xt_all[:, tb, c, :],
                            start=(c == 0), stop=(c == NCC - 1),
                        )
                    nc.vector.tensor_scalar_add(
                        kT_t[m][:, bass.ts(tb, 512)], ps, bk_col[:, m: m + 1]
                    )
                return emit

            def v_unit(tb, tsub):
                def emit():
                    kc = tb * 4 + tsub
                    ps = fpool.tile([128, 512], FP32, tag="fps", name="fps")
                    for c in range(NCC):
                        nc.tensor.matmul(
                            ps, xt_all[:, tb, c, bass.ts(tsub, 128)],
                            wv_all[:, c, :],
                            start=(c == 0), stop=(c == NCC - 1),
                        )
                    nc.vector.tensor_add(
                        v_pad[:, kc, :, 0:HD],
                        ps.rearrange("p (h d) -> p h d", h=HPG),
                        bv_bc.rearrange("p (h d) -> p h d", h=HPG),
                    )
                return emit

            proj_osb = {}

            def proj_unit(qb, tsub, nb, split_dma=False):
                def emit():
                    tb16 = qb * 4 + tsub
                    ps = fpool.tile([128, 512], FP32, tag="fps", name="fps")
                    for c in range(NMB):
                        nc.tensor.matmul(
                            ps, yT_t[c][:, bass.ts(tb16, 128)],
                            wp_all[:, c, bass.ts(nb, 512)],
                            start=(c == 0), stop=(c == NMB - 1),
                        )
                    if split_dma:
                        osb = osbpool.tile([128, 512], FP32, tag="osbh",
                                           name="osbh")
                        nc.vector.tensor_copy(osb, ps)
                        nc.sync.dma_start(
                            out=out[bass.ts(tb16, 128), bass.ts(nb, 512)],
                            in_=osb,
                        )
                    else:
                        if nb == 0:
                            proj_osb[tb16] = osbpool.tile(
                                [128, C], FP32, tag="osb", name="osb")
                        osb = proj_osb[tb16]
                        nc.vector.tensor_copy(osb[:, bass.ts(nb, 512)], ps)
                        if nb == 1:
                            nc.sync.dma_start(
                                out=out[bass.ts(tb16, 128), :], in_=osb
                            )
                return emit

            def qkv_units(tb):
                us = []
                for m in range(NMB):
                    us.append(q_unit(tb, m))
                for m in range(NMB):
                    us.append(k_unit(tb, m))
                for tsub in range(4):
                    us.append(v_unit(tb, tsub))
                return us

            def proj_units(qb, split_dma=False):
                return [proj_unit(qb, tsub, nb, split_dma)
                        for tsub in range(4) for nb in range(2)]

            # ---------------- attention groups -----------------------------
            def chunk_geom(qb, kc):
                c_off = kc - 4 * qb
                w = 512 if c_off < 0 else 512 - 128 * c_off
                return c_off, 512 - w          # (diag offset, first live col)

            def attn_window(groups, units, flush_before=None):
                """groups: list of (qb, m).  units: filler closures, pumped
                evenly across halfsteps.  flush_before: {group_idx: n} force-
                flushes the first n units before that group starts."""
                flush_before = flush_before or {}
                n_hs = sum(4 * (qb + 1) for qb, _ in groups)
                state = {"u": 0, "hs": 0}

                def pump_to(k):
                    while state["u"] < min(k, len(units)):
                        units[state["u"]]()
                        state["u"] += 1

                def pump():
                    state["hs"] += 1
                    while (state["u"] < len(units)
                           and state["u"] * n_hs < state["hs"] * len(units)):
                        units[state["u"]]()
                        state["u"] += 1

                for gi, (qb, m) in enumerate(groups):
                    if gi in flush_before:
                        pump_to(flush_before[gi])
                    P = 2 * (qb + 1)
                    kcmax = 4 * (qb + 1)
                    pv = [pvpool.tile([128, 512], FP32, tag="pv", name=f"pv{_hp}")
                          for _hp in range(2)]

                    def qk_exp(hp, p):
                        S = spool.tile([128, 2, 512], FP32, tag="sp", name="S")
                        pcol0 = 512
                        for j in range(2):
                            kc = 2 * p + j
                            c_off, col0 = chunk_geom(qb, kc)
                            pcol0 = min(pcol0, col0)
                            diag = c_off >= 0
                            nc.tensor.matmul(
                                S[:, j, col0:512],
                                kT_t[m][:, bass.ts(kc, 128)],
                                qTp[hp][m][:, qb * 512 + col0: qb * 512 + 512],
                                start=True, stop=not diag,
                            )
                            if diag:
                                nc.tensor.matmul(
                                    S[:, j, col0: col0 + 128],
                                    ident, tri_neg,
                                    start=False, stop=True,
                                )
                        pt = ptpool.tile([128, 2, 512], BF16, tag="pt", name="pt")
                        nc.scalar.activation(
                            out=pt[:, :, pcol0:512],
                            in_=S[:, :, pcol0:512],
                            func=mybir.ActivationFunctionType.Exp,
                            scale=1.0 / float(np.sqrt(HD)),
                        )
                        return pt

                    def emit_pv(hp, p, pt):
                        for j in range(2):
                            kc = 2 * p + j
                            _, col0 = chunk_geom(qb, kc)
                            nc.tensor.matmul(
                                pv[hp][0:VP, col0:512],
                                v_pad[:, kc, 2 * m + hp, :],
                                pt[:, j, col0:512],
                                start=(kc == 0), stop=(kc == kcmax - 1),
                            )

                    def drain_norm(hp):
                        nc.vector.tensor_copy(
                            yT_t[m][bass.ts(hp, 64), bass.ts(qb, 512)],
                            pv[hp][0:HD, :],
                        )
                        den = dnpool.tile([1, 512], FP32, tag=f"den{hp}",
                                          name=f"den{hp}")
                        nc.vector.tensor_copy(den, pv[hp][HD: HD + 1, :])
                        denr = dnpool.tile([1, 512], FP32, tag=f"denr{hp}",
                                           name=f"denr{hp}")
                        nc.vector.reciprocal_approx_fast(denr, den)
                        rbc = rbcpool.tile([128, 512], FP32, tag="rbc",
                                           name="rbc")
                        nc.gpsimd.partition_broadcast(rbc, denr)
                        yt = yT_t[m][bass.ts(hp, 64), bass.ts(qb, 512)]
                        nc.vector.tensor_mul(yt, yt, rbc[bass.ts(hp, 64), :])

                    prev = [None, None]
                    for p in range(P):
                        pt_a = qk_exp(0, p)
                        if p > 0:
                            emit_pv(1, p - 1, prev[1])
                        pump()
                        pt_b = qk_exp(1, p)
                        if p > 0:
                            emit_pv(0, p - 1, prev[0])
                        pump()
                        prev = [pt_a, pt_b]
                    emit_pv(0, P - 1, prev[0])
                    drain_norm(0)
                    emit_pv(1, P - 1, prev[1])
                    drain_norm(1)

                pump_to(len(units))

            # ---------------- schedule -------------------------------------
            for u in qkv_units(0):
                u()
            attn_window([(0, m) for m in range(NMB)], qkv_units(1))
            attn_window([(1, m) for m in range(NMB)], qkv_units(2))
            # merged windows 2+3: qb3 m-groups interleave with qb2's.
            # filler: qkv(3) first (q units, then k/v), then proj(0..2).
            w23_units = qkv_units(3) + proj_units(0) + proj_units(1) \
                + proj_units(2)
            w23_groups = [(2, 0), (2, 1), (2, 2), (3, 0),
                          (2, 3), (3, 1), (3, 2), (3, 3)]
            # qkv(3) (first 12 units) must be done before any qb3 group
            attn_window(w23_groups, w23_units, flush_before={3: 12})
            for u in proj_units(3, split_dma=True):
                u()

    nc.compile()
    return nc


_CACHE = {}


def _get_nc():
    if "nc" not in _CACHE:
        _CACHE["nc"] = build_nc()
    return _CACHE["nc"]


def _to_bf16(a):
    import ml_dtypes

    return np.ascontiguousarray(np.asarray(a, dtype=np.float32).astype(ml_dtypes.bfloat16))


def make_in_maps(x, w_qkv, b_qkv, w_proj):
    x = np.asarray(x, dtype=np.float32)
    w_qkv = np.asarray(w_qkv, dtype=np.float32)
    b_qkv = np.asarray(b_qkv, dtype=np.float32)
    w_proj = np.asarray(w_proj, dtype=np.float32)

    # x[b]^T tiled: xt4[d, tb, c, tw] = x[b][512*tb+tw, 128*c+d]
    xt4s = [
        _to_bf16(x[b].T.reshape(NCC, 128, NTB, 512).transpose(1, 2, 0, 3))
        for b in range(B)
    ]

    def w_tiles(w):  # [C, N] -> [128, C//128, N]
        n = w.shape[1]
        return _to_bf16(w.reshape(C // 128, 128, n).transpose(1, 0, 2))

    def wp_tiles(w):  # [GD, C] -> [128, GD//128, C]
        return _to_bf16(w.reshape(GD // 128, 128, C).transpose(1, 0, 2))

    in_maps = []
    for core in range(N_CORES):
        b, g = divmod(core, G)
        gs = slice(GD * g, GD * g + GD)
        bq = b_qkv[gs]
        bk = b_qkv[C + GD * g: C + GD * g + GD]
        bv = b_qkv[2 * C + GD * g: 2 * C + GD * g + GD]
        in_maps.append(
            {
                "xt4": xt4s[b],
                "wq": w_tiles(w_qkv[:, gs]),
                "wk": w_tiles(w_qkv[:, C + GD * g: C + GD * g + GD]),
                "wv": w_tiles(w_qkv[:, 2 * C + GD * g: 2 * C + GD * g + GD]),
                "bqc": np.ascontiguousarray(bq.reshape(NMB, 128).T),
                "bkc": np.ascontiguousarray(bk.reshape(NMB, 128).T),
                "bvb": np.ascontiguousarray(
                    np.broadcast_to(bv, (128, GD)).copy()),
                "wp": wp_tiles(w_proj[gs, :]),
            }
        )
    return in_maps


def _assemble(results, b_proj):
    y = np.empty((B, T, C), dtype=np.float32)
    for b in range(B):
        y[b] = results[G * b]["out"] + results[G * b + 1]["out"]
    y += np.asarray(b_proj, dtype=np.float32)[None, None, :]
    return y


def kernel(x, w_qkv, b_qkv, w_proj, b_proj):
    nc = _get_nc()
    in_maps = make_in_maps(x, w_qkv, b_qkv, w_proj)
    res = run_bass_kernel_spmd(nc, in_maps, list(range(N_CORES)))
    return _assemble(res.results, b_proj)
